# revision 21
# baseline (speedup 1.0000x reference)
"""GCBlock GNN message-passing kernel for 8 Trainium2 NeuronCores.

Strategy:
  * Host: sort edges by destination idx_i, shard at node boundaries into 8
    balanced slices (each core owns a disjoint output node range -> no
    collectives), pack edges into 128-edge tiles that never split a node,
    fold pi_w2 @ ii_w1 into a single W_mid (no nonlinearity between them).
  * Device phase A: every core computes the full pp1 = MLP(p1) node table
    into a DRAM scratch (feature-major matmuls, tanh on ScalarE).
  * Device phase B (per 512-edge chunk): indirect-DMA gather pp1 rows for
    idx_i/idx_j, DVE add, PE transposes into PSUM, add host-pre-transposed
    basis, 3 matmul layers (fp32r), tanh on ScalarE, one-hot scatter
    matmuls into a 32-node window PSUM, indirect-DMA scatter of window rows
    to the output slice (each node lives in exactly one tile -> plain
    race-free writes).
"""

import math

import numpy as np

import concourse.bacc as bacc
import concourse.bass as bass
import concourse.mybir as mybir
from concourse.bass import IndirectOffsetOnAxis
from concourse.bass_utils import run_bass_kernel_spmd
from concourse.tile import TileContext

D = 64
TILE = 128          # edges per tile
TPC = 4             # tiles per chunk
CHUNK = TILE * TPC  # 512 edges/nodes per chunk
WIN = 32            # scatter window rows per tile
NCORES = 8
PAD_LOC = 300.0     # one-hot local index for pad edges (matches nothing)

FP = mybir.dt.float32
FR = mybir.dt.float32r
NPF = np.float32

USE_BF16 = True
BF = mybir.dt.bfloat16
if USE_BF16:
    import ml_dtypes
    NPB = ml_dtypes.bfloat16
    DT = BF
    NPD = NPB
else:
    DT = FP
    NPD = NPF

# tensors that move to bf16 when USE_BF16 (host side)
BF_CONSTS = ["p1t", "w1pp", "w2pp", "w1pi", "wmid", "w2ii", "ident", "iota",
             "ones_row", "bpp2_row", "bii2_row"]
BF_PER_CORE = ["basis_p", "locf"]


def _table_row(g):
    """Physical row of node g in the packed pp1 table ([rows, 64] view)."""
    return (g // CHUNK) * 512 + (g % 128) * 4 + (g % CHUNK) // 128


# ---------------------------------------------------------------- host prep

def _pack_fm(tiles_em):
    """tiles_em: [4, 128, 64] edge-major tiles -> [64, 512] FM."""
    out = np.zeros((64, 512), dtype=NPF)
    for k in range(TPC):
        out[:, 128 * k:128 * k + 128] = tiles_em[k].T
    return out


def prep(idx_i, idx_j, p1, basis, weights):
    N, E = p1.shape[0], idx_i.shape[0]
    NA = math.ceil(N / CHUNK)

    order = np.argsort(idx_i, kind="stable")
    si = idx_i[order]
    sj = idx_j[order]
    sb = basis[order]

    # core boundaries snapped to node edges, balancing edge counts
    node_bounds = [0]
    edge_bounds = [0]
    for c in range(1, NCORES):
        pos = min(int(round(c * E / NCORES)), E - 1)
        node_c = max(int(si[pos]), node_bounds[-1] + 1)
        node_bounds.append(node_c)
        edge_bounds.append(int(np.searchsorted(si, node_c)))
    node_bounds.append(N)
    edge_bounds.append(E)

    # per-core tile packing (no node spans two tiles; window spread < WIN)
    core_tiles = []
    for c in range(NCORES):
        s, e = edge_bounds[c], edge_bounds[c + 1]
        nb = node_bounds[c]
        loc_nodes = si[s:e] - nb
        nsl = node_bounds[c + 1] - nb
        deg = np.bincount(loc_nodes, minlength=nsl)
        nz = np.flatnonzero(deg)
        node_estart = s + np.concatenate([[0], np.cumsum(deg)[:-1]])
        firsts, lasts, estarts, ecounts = [], [], [], []
        cur_first = None
        for n in nz:
            d = int(deg[n])
            assert d <= TILE, f"node degree {d} > {TILE} unsupported"
            if cur_first is None or cur_cnt + d > TILE or n - cur_first >= WIN:
                if cur_first is not None:
                    firsts.append(cur_first)
                    lasts.append(cur_last)
                    estarts.append(cur_es)
                    ecounts.append(cur_cnt)
                cur_first, cur_cnt, cur_es = int(n), 0, int(node_estart[n])
            cur_cnt += d
            cur_last = int(n)
        if cur_first is not None:
            firsts.append(cur_first)
            lasts.append(cur_last)
            estarts.append(cur_es)
            ecounts.append(cur_cnt)
        core_tiles.append((firsts, lasts, estarts, ecounts))

    NT = max(len(t[0]) for t in core_tiles)
    NCHUNK = math.ceil(NT / TPC)
    NT = NCHUNK * TPC
    NSL = max(node_bounds[c + 1] - node_bounds[c] for c in range(NCORES))
    DUMP = NSL
    NBLKF = math.ceil((NSL + 1) / 128)

    per_core = []
    for c in range(NCORES):
        firsts, lasts, estarts, ecounts = core_tiles[c]
        nb = node_bounds[c]
        basis_p = np.zeros((NCHUNK, 64, 512), dtype=NPF)
        gidx = np.zeros((NCHUNK, 128, TPC), dtype=np.int32)
        gjdx = np.zeros((NCHUNK, 128, TPC), dtype=np.int32)
        locf = np.full((NCHUNK, 128, TPC), PAD_LOC, dtype=NPF)
        scat = np.full((NCHUNK, WIN, TPC), DUMP, dtype=np.int32)
        tiles_em = np.zeros((TPC, 128, D), dtype=NPF)
        for ch in range(NCHUNK):
            tiles_em[:] = 0.0
            for k in range(TPC):
                t = ch * TPC + k
                if t >= len(firsts):
                    continue
                es, cnt, fn, ln = estarts[t], ecounts[t], firsts[t], lasts[t]
                tiles_em[k, :cnt] = sb[es:es + cnt]
                gidx[ch, :cnt, k] = si[es:es + cnt]
                gjdx[ch, :cnt, k] = sj[es:es + cnt]
                locf[ch, :cnt, k] = (si[es:es + cnt] - nb - fn).astype(NPF)
                nrows = ln - fn + 1
                scat[ch, :nrows, k] = np.arange(fn, ln + 1)
            basis_p[ch] = _pack_fm(tiles_em)
        gidx = _table_row(gidx.astype(np.int64)).astype(np.int32)
        gjdx = _table_row(gjdx.astype(np.int64)).astype(np.int32)
        # final-pass compaction: output row n <- stage row 32*t + (n - first_t)
        fidx = np.zeros((NBLKF * 128,), dtype=np.int32)
        for t in range(len(firsts)):
            fn, ln = firsts[t], lasts[t]
            fidx[fn:ln + 1] = t * WIN + np.arange(ln + 1 - fn)
        fidx = fidx.reshape(NBLKF, 128, 1)
        per_core.append(dict(basis_p=basis_p, gidx=gidx, gjdx=gjdx,
                             locf=locf, scat=scat, fidx=fidx))

    # phase A packing (same for all cores)
    p1_pad = np.zeros((NA * CHUNK, D), dtype=NPF)
    p1_pad[:N] = p1
    p1t = np.zeros((NA, 64, 512), dtype=NPF)
    for a in range(NA):
        p1t[a] = _pack_fm(p1_pad[a * CHUNK:(a + 1) * CHUNK].reshape(TPC, 128, D))

    w = weights
    W_mid = (w["pi_w2"] @ w["ii_w1"]).astype(NPF)
    b_mid = (w["pi_b2"] @ w["ii_w1"] + w["ii_b1"]).astype(NPF)

    consts = dict(
        p1t=p1t,
        w1pp=w["pp_w1"].astype(NPF), w2pp=w["pp_w2"].astype(NPF),
        w1pi=w["pi_w1"].astype(NPF), wmid=W_mid,
        w2ii=w["ii_w2"].astype(NPF),
        ident=np.eye(128, dtype=NPF),
        iota=np.tile(np.arange(WIN, dtype=NPF), (128, 1)),
        b_pp1=w["pp_b1"].reshape(64, 1).astype(NPF),
        b_pi1=w["pi_b1"].reshape(64, 1).astype(NPF),
        b_mid=b_mid.reshape(64, 1),
        ones_row=np.ones((1, 128), dtype=NPF),
        bpp2_row=w["pp_b2"].reshape(1, D).astype(NPF),
        bii2_row=w["ii_b2"].reshape(1, D).astype(NPF),
    )
    if USE_BF16:
        for nm in BF_CONSTS:
            consts[nm] = consts[nm].astype(NPB)
        for pc in per_core:
            for nm in BF_PER_CORE:
                pc[nm] = pc[nm].astype(NPB)

    dims = dict(N=N, E=E, NA=NA, NCHUNK=NCHUNK, NSL=NSL, NBLKF=NBLKF,
                node_bounds=node_bounds)
    return per_core, consts, dims


# ------------------------------------------------------------- device build

def build(nc, dims, consts, sections=("A", "B")):
    import os
    _NOGATHER = bool(os.environ.get("GC_NOGATHER"))
    NA, NCHUNK, NSL = dims["NA"], dims["NCHUNK"], dims["NSL"]
    has_bpp2 = bool(np.any(consts["bpp2_row"] != 0))
    has_bii2 = bool(np.any(consts["bii2_row"] != 0))
    has_bpp1 = bool(np.any(consts["b_pp1"] != 0))
    has_bpi1 = bool(np.any(consts["b_pi1"] != 0))
    has_bmid = bool(np.any(consts["b_mid"] != 0))

    t_p1t = nc.dram_tensor("p1t", (NA, 64, 512), DT, kind="ExternalInput")
    t_basis = nc.dram_tensor("basis_p", (NCHUNK, 64, 512), DT, kind="ExternalInput")
    t_gidx = nc.dram_tensor("gidx", (NCHUNK, 128, TPC), mybir.dt.int32, kind="ExternalInput")
    t_gjdx = nc.dram_tensor("gjdx", (NCHUNK, 128, TPC), mybir.dt.int32, kind="ExternalInput")
    t_locf = nc.dram_tensor("locf", (NCHUNK, 128, TPC), DT, kind="ExternalInput")
    t_fidx = nc.dram_tensor("fidx", (dims["NBLKF"], 128, 1), mybir.dt.int32, kind="ExternalInput")
    cts = {}
    cdt = {}
    for nm in ["w1pp", "w2pp", "w1pi", "wmid", "w2ii", "ident", "iota",
               "b_pp1", "b_pi1", "b_mid", "ones_row", "bpp2_row", "bii2_row"]:
        cdt[nm] = DT if (USE_BF16 and nm in BF_CONSTS) else FP
        cts[nm] = nc.dram_tensor(nm, consts[nm].shape, cdt[nm], kind="ExternalInput")
    NBLKF = dims["NBLKF"]
    t_out = nc.dram_tensor("out", (NBLKF * 128, D), FP, kind="ExternalOutput")
    table = nc.dram_tensor("pp1_table", (NA * 128, 256), DT, kind="Internal")
    stage = nc.dram_tensor("stage", (NCHUNK * TPC * WIN, D), FP, kind="Internal")
    table_rows = table[:].rearrange("r (k f) -> (r k) f", k=TPC)  # [NA*512, 64]

    def load_consts(pool):
        sb = {}
        for nm, t in cts.items():
            tile = pool.tile(list(consts[nm].shape), cdt[nm], tag=nm)
            nc.sync.dma_start(tile[:], t[:])
            sb[nm] = tile
        return sb

    Tanh = mybir.ActivationFunctionType.Tanh
    Copy = mybir.ActivationFunctionType.Copy

    def mm(out, lhsT, rhs, start=True, stop=True):
        nc.tensor.matmul(out, lhsT=lhsT, rhs=rhs, start=start, stop=stop)

    # EM layer: psum [128, 256] col-block k <- h[:, 128k:+128].T @ w (+ bias)
    def em_layer(ps, h, w_sb, bias_row, has_bias, sbk):
        for k in range(TPC):
            mm(ps[:, 64 * k:64 * k + 64], h[:, 128 * k:128 * k + 128],
               w_sb[:], start=True, stop=not has_bias)
            if has_bias:
                mm(ps[:, 64 * k:64 * k + 64], sbk["ones_row"][:, :],
                   bias_row[:, :], start=False, stop=True)

    # ---------------- phase A: pp1 table ----------------
    na = NA if "A" in sections else 1
    with TileContext(nc) as tc:
        with tc.tile_pool(name="cst", bufs=1) as cpool, \
             tc.tile_pool(name="sba", bufs=3) as pool, \
             tc.tile_pool(name="psa", bufs=2, space="PSUM") as pspool:
            sbk = load_consts(cpool)
            for a in range(na):
                p1c = pool.tile([64, 512], DT, tag="p1c")
                nc.sync.dma_start(p1c[:], t_p1t[a])
                ps1 = pspool.tile([64, 512], FP, tag="ps1")
                mm(ps1[:], sbk["w1pp"][:], p1c[:])
                h1 = pool.tile([64, 512], DT, tag="h1a")
                if has_bpp1:
                    nc.scalar.activation(h1[:], ps1[:], Tanh, bias=sbk["b_pp1"][:])
                else:
                    nc.scalar.activation(h1[:], ps1[:], Tanh)
                ps2 = pspool.tile([128, 256], FP, tag="ps2")
                em_layer(ps2, h1, sbk["w2pp"], sbk["bpp2_row"], has_bpp2, sbk)
                pe = pool.tile([128, 256], DT, tag="pea")
                nc.vector.tensor_copy(pe[:], ps2[:])
                nc.sync.dma_start(table[a * 128:(a + 1) * 128, :], pe[:])

    # ---------------- phase B: edges ----------------
    nch = NCHUNK if "B" in sections else 0
    with TileContext(nc) as tc:
        with tc.tile_pool(name="cstb", bufs=1) as cpool, \
             tc.tile_pool(name="sbb", bufs=4) as pool, \
             tc.tile_pool(name="meta", bufs=4) as mpool, \
             tc.tile_pool(name="psI", bufs=2, space="PSUM") as psI, \
             tc.tile_pool(name="psH", bufs=1, space="PSUM") as psH, \
             tc.tile_pool(name="psE", bufs=1, space="PSUM") as psE, \
             tc.tile_pool(name="psS", bufs=2, space="PSUM") as psS:
            sbk = load_consts(cpool)
            for ch in range(nch):
                bas = pool.tile([64, 512], DT, tag="bas")
                nc.sync.dma_start(bas[:], t_basis[ch])
                gi_sb = mpool.tile([128, TPC], mybir.dt.int32, tag="gi")
                nc.sync.dma_start(gi_sb[:], t_gidx[ch])
                gj_sb = mpool.tile([128, TPC], mybir.dt.int32, tag="gj")
                nc.sync.dma_start(gj_sb[:], t_gjdx[ch])
                loc_sb = mpool.tile([128, TPC], DT, tag="loc")
                nc.sync.dma_start(loc_sb[:], t_locf[ch])
                graw = pool.tile([128, 256], DT, tag="graw")
                gjraw = pool.tile([128, 256], DT, tag="gjraw")
                if _NOGATHER:
                    r0 = (ch % NA) * 128
                    nc.sync.dma_start(graw[:], table[r0:r0 + 128, :])
                    nc.sync.dma_start(gjraw[:], table[r0:r0 + 128, :])
                else:
                    for k in range(TPC):
                        nc.gpsimd.indirect_dma_start(
                            out=graw[:, 64 * k:64 * k + 64], out_offset=None,
                            in_=table_rows,
                            in_offset=IndirectOffsetOnAxis(ap=gi_sb[:, k:k + 1], axis=0))
                        nc.gpsimd.indirect_dma_start(
                            out=gjraw[:, 64 * k:64 * k + 64], out_offset=None,
                            in_=table_rows,
                            in_offset=IndirectOffsetOnAxis(ap=gj_sb[:, k:k + 1], axis=0))
                gsum = pool.tile([128, 256], DT, tag="gsum")
                nc.vector.tensor_tensor(out=gsum[:], in0=graw[:], in1=gjraw[:],
                                        op=mybir.AluOpType.add)

                psi = psI.tile([64, 512], DT, tag="psi")
                for k in range(TPC):
                    nc.tensor.matmul(psi[:, 128 * k:128 * k + 128],
                                     lhsT=gsum[:, 64 * k:64 * k + 64],
                                     rhs=sbk["ident"][:], is_transpose=True,
                                     start=True, stop=True)
                interf = pool.tile([64, 512], DT, tag="interf")
                nc.vector.tensor_tensor(out=interf[:], in0=psi[:], in1=bas[:],
                                        op=mybir.AluOpType.add)

                ph1 = psH.tile([64, 512], FP, tag="ph1")
                mm(ph1[:], sbk["w1pi"][:], interf[:])
                h1 = pool.tile([64, 512], DT, tag="h1")
                if has_bpi1:
                    nc.scalar.activation(h1[:], ph1[:], Tanh, bias=sbk["b_pi1"][:])
                else:
                    nc.scalar.activation(h1[:], ph1[:], Tanh)

                ph2 = psH.tile([64, 512], FP, tag="ph2")
                mm(ph2[:], sbk["wmid"][:], h1[:])
                h2 = pool.tile([64, 512], DT, tag="h2")
                if has_bmid:
                    nc.scalar.activation(h2[:], ph2[:], Tanh, bias=sbk["b_mid"][:])
                else:
                    nc.scalar.activation(h2[:], ph2[:], Tanh)

                pse = psE.tile([128, 256], FP, tag="pse")
                em_layer(pse, h2, sbk["w2ii"], sbk["bii2_row"], has_bii2, sbk)
                iiem = pool.tile([128, 256], DT, tag="iiem")
                nc.scalar.activation(iiem[:], pse[:], Copy)

                pss = psS.tile([WIN, 256], FP, tag="pss")
                for k in range(TPC):
                    oh = mpool.tile([128, WIN], DT, tag=f"oh{k % 2}")
                    nc.vector.tensor_tensor(
                        out=oh[:],
                        in0=loc_sb[:, k:k + 1].to_broadcast([128, WIN]),
                        in1=sbk["iota"][:, :],
                        op=mybir.AluOpType.is_equal)
                    mm(pss[:, 64 * k:64 * k + 64], oh[:],
                       iiem[:, 64 * k:64 * k + 64])
                s_sb = pool.tile([WIN, 256], FP, tag="s_sb")
                nc.vector.tensor_copy(s_sb[:], pss[:])
                st = stage[ch * TPC * WIN:(ch + 1) * TPC * WIN, :]
                nc.sync.dma_start(
                    st.rearrange("(k p) f -> p k f", k=TPC),
                    s_sb[:].rearrange("p (k f) -> p k f", k=TPC))
    # ---------------- phase C: compact stage -> out ----------------
    with TileContext(nc) as tc:
        with tc.tile_pool(name="sbc", bufs=4) as pool, \
             tc.tile_pool(name="metac", bufs=4) as mpool:
            for b in range(NBLKF if "B" in sections else 0):
                fx = mpool.tile([128, 1], mybir.dt.int32, tag="fx")
                nc.sync.dma_start(fx[:], t_fidx[b])
                g = pool.tile([128, D], FP, tag="g")
                nc.gpsimd.indirect_dma_start(
                    out=g[:], out_offset=None, in_=stage[:],
                    in_offset=IndirectOffsetOnAxis(ap=fx[:], axis=0))
                nc.sync.dma_start(t_out[b * 128:(b + 1) * 128, :], g[:])
    nc.compile()


# ----------------------------------------------------------------- kernel()

SHARED_NAMES = ["w1pp", "w2pp", "w1pi", "wmid", "w2ii", "ident", "iota",
                "b_pp1", "b_pi1", "b_mid", "ones_row", "bpp2_row",
                "bii2_row", "p1t"]
PER_CORE_NAMES = ["basis_p", "gidx", "gjdx", "locf", "fidx"]


def make_in_maps(per_core, consts):
    shared = {nm: consts[nm] for nm in SHARED_NAMES}
    in_maps = []
    for c in range(NCORES):
        m = dict(shared)
        for nm in PER_CORE_NAMES:
            m[nm] = per_core[c][nm]
        in_maps.append(m)
    return in_maps


def kernel(**inputs):
    idx_i = np.asarray(inputs["idx_i"]).astype(np.int64)
    idx_j = np.asarray(inputs["idx_j"]).astype(np.int64)
    p1 = np.asarray(inputs["p1"], dtype=NPF)
    basis = np.asarray(inputs["basis"], dtype=NPF)
    weights = {k: np.asarray(inputs[k], dtype=NPF) for k in
               ["pp_w1", "pp_b1", "pp_w2", "pp_b2",
                "pi_w1", "pi_b1", "pi_w2", "pi_b2",
                "ii_w1", "ii_b1", "ii_w2", "ii_b2"]}

    per_core, consts, dims = prep(idx_i, idx_j, p1, basis, weights)

    nc = bacc.Bacc(trn_type="TRN2")
    build(nc, dims, consts)

    import os
    trace = bool(os.environ.get("GC_TRACE"))
    res = run_bass_kernel_spmd(nc, make_in_maps(per_core, consts),
                               core_ids=list(range(NCORES)), trace=trace)
    global LAST_EXEC_NS
    LAST_EXEC_NS = res.exec_time_ns

    N = dims["N"]
    nbs = dims["node_bounds"]
    out = np.zeros((N, D), dtype=NPF)
    for c in range(NCORES):
        out[nbs[c]:nbs[c + 1]] = res.results[c]["out"][:nbs[c + 1] - nbs[c]]
    deg = np.bincount(idx_i, minlength=N)
    out[deg == 0] = 0
    return out
